# revision 1
# baseline (speedup 1.0000x reference)
"""GAT 2-layer model on 8 Trainium2 NeuronCores (Bass/Tile) — v2.

Strategy (vs v1): nodes dst-sharded across 8 cores; edges of core p grouped
by (dst-window of 128, src-block of 25k), cell-packed into C chunks of 128
slots with padding at each cell's tail. fc1 replicated: each core computes
the full [N,272] fc1+att table in its HBM (t1, 768B bf16 rows: 256 feat |
8 f32 el | pad) plus a compact er table (er1f). Per window: one dma_gather
per src-block with NEGATIVE trailing pad indices (descriptors skipped);
el comes with the gathered rows; er[dst] delivered on-chip: a transposed
one-hot stT (built on DVE from a host-packed broadcast dmod table) feeds
tiny PE matmuls that scatter the window's 128 er values to edge slots —
no per-edge er gather, no er_loc tables. leaky-relu+exp done as
max(exp(v), exp(0.2 v)) so ACT only ever runs Exp (no act-table reloads);
relu on DVE; sigmoid once at the end. Segment-sum via one-hot matmul into
PSUM. conv2 repeats the machinery on the AllGathered t2 table.
"""
import sys

for _p in ("/opt/trn_rl_repo",):
    if _p not in sys.path:
        sys.path.insert(0, _p)

import math
from dataclasses import dataclass

import numpy as np
import ml_dtypes

import concourse.bass as bass
import concourse.bacc as bacc
import concourse.mybir as mybir
import concourse.tile as tile
from concourse.bass_utils import run_bass_kernel_spmd

BF16 = ml_dtypes.bfloat16
NCORES = 8
NQ = 4  # src blocks
NEG = 0.2

IN_F = 128
HID = 32
HEADS = 8
OUT_F = 64
F1 = HEADS * HID          # 256
T1W = 384                 # bf16 cols per T1 row: 256 feat | 16 (el f32) | pad
T2W = 128                 # bf16 cols per T2 row: 64 feat | 2 (el2 f32) | pad
SCRATCH = 16384           # dynamic DMA scratch (default)
NQUEUES = 4


@dataclass(frozen=True)
class Cfg:
    n: int
    e: int
    c: int  # slot chunks (x128) per (window, src-block) cell

    @property
    def ln(self):
        return self.n // NCORES

    @property
    def nw(self):
        return math.ceil(self.ln / 128)

    @property
    def t2ln(self):
        return self.nw * 128

    @property
    def blkn(self):
        return self.n // NQ

    @property
    def t2blk(self):
        return 2 * self.t2ln

    @property
    def npad(self):
        return 1024 * math.ceil(self.n / 1024)

    @property
    def nbatch(self):
        return self.npad // 1024

    @property
    def ic(self):  # int16 idx cols per (window, q) cell
        return self.c * 128 // 16

    @property
    def pkt_w(self):  # int16 cols of the per-window packet
        # idx1[NQ][ic] | idx2[NQ][ic] | dmod[NQ*C] | X[NQ*C*128]
        w = 2 * NQ * self.ic + NQ * self.c + NQ * self.c * 128
        return 16 * math.ceil(w / 16)

    @property
    def off_idx2(self):
        return NQ * self.ic

    @property
    def off_dmod(self):
        return 2 * NQ * self.ic

    @property
    def off_x(self):
        return 2 * NQ * self.ic + NQ * self.c


def _fold(al, heads, hid):
    a = np.zeros((heads * hid, heads), np.float32)
    for h in range(heads):
        a[h * hid:(h + 1) * hid, h] = al[h]
    return a


def _wrap_idx(flat):
    """[n] int -> [128, n//16] int16: wrapped in 16 partitions, replicated 8x."""
    n = len(flat)
    w = np.asarray(flat, np.int16).reshape(n // 16, 16).T
    return np.tile(w, (8, 1))


def prep(inputs, cfg: Cfg | None = None):
    """Host-side: fold weights, pack per-core edge schedules."""
    feats = np.asarray(inputs["features"], np.float32)
    src = np.asarray(inputs["src"], np.int64)
    dst = np.asarray(inputs["dst"], np.int64)
    n, e = feats.shape[0], src.shape[0]

    W1f = np.concatenate(
        [inputs["W1"],
         inputs["W1"] @ _fold(np.asarray(inputs["al1"]), HEADS, HID),
         inputs["W1"] @ _fold(np.asarray(inputs["ar1"]), HEADS, HID)], axis=1
    ).astype(np.float32)  # [128, 272]
    W2f = np.concatenate(
        [inputs["W2"],
         inputs["W2"] @ _fold(np.asarray(inputs["al2"]), 1, OUT_F),
         inputs["W2"] @ _fold(np.asarray(inputs["ar2"]), 1, OUT_F)], axis=1
    ).astype(np.float32)  # [256, 66]

    ln = n // NCORES
    core = dst // ln
    nw = math.ceil(ln / 128)
    blkn = n // NQ
    w_of = (dst - core * ln) // 128
    q_of = src // blkn
    cell = ((core * nw + w_of) * NQ + q_of).astype(np.int64)
    counts = np.bincount(cell, minlength=NCORES * nw * NQ)
    c_need = math.ceil(counts.max() / 128)
    if cfg is None:
        cfg = Cfg(n=n, e=e, c=int(c_need))
    assert counts.max() <= cfg.c * 128, (counts.max(), cfg.c)
    C = cfg.c

    t2row_src = (src // ln) * cfg.t2ln + (src % ln)

    order = np.lexsort((q_of, w_of, core))
    src_s, dst_s = src[order], dst[order]
    core_s, w_s, q_s = core[order], w_of[order], q_of[order]
    t2src_s = t2row_src[order]

    in_maps = []
    featT = np.zeros((128, cfg.npad), BF16)
    featT[:, :n] = feats.T.astype(BF16)
    iota_rep = np.tile(np.arange(128, dtype=np.float32), (128, 1)).astype(BF16)
    iota_p = np.arange(128, dtype=np.float32)[:, None].astype(BF16)  # [128,1]
    ident = np.eye(128, dtype=np.float32).astype(BF16)
    w1f_b = W1f.astype(BF16)
    w2f_b = np.zeros((128, 2, 66), BF16)
    w2f_b[:, 0, :] = W2f[:128].astype(BF16)
    w2f_b[:, 1, :] = W2f[128:].astype(BF16)
    b1_rep = np.tile(np.asarray(inputs["b1"], np.float32)[None, :], (128, 1))
    b2_rep = np.tile(np.asarray(inputs["b2"], np.float32)[None, :], (128, 1))
    pW_rep = np.tile(np.asarray(inputs["pW"], np.float32)[:, 0][None, :], (128, 1))
    pb_t = np.full((128, 1), float(np.asarray(inputs["pb"])[0]), np.float32)

    ic = cfg.ic
    for p in range(NCORES):
        sel = core_s == p
        es, ed, ew, eq, et2 = src_s[sel], dst_s[sel], w_s[sel], q_s[sel], t2src_s[sel]
        dl = ed - p * ln
        s1 = np.zeros((nw, NQ, C * 128), np.int16)
        s2 = np.zeros((nw, NQ, C * 128), np.int16)
        dm = np.full((nw, NQ, C * 128), 200.0, np.float32)
        ord2 = np.lexsort((eq, ew))
        ew2, eq2 = ew[ord2], eq[ord2]
        cellid = ew2 * NQ + eq2
        pos = np.arange(len(cellid)) - np.concatenate(
            ([0], np.cumsum(np.bincount(cellid, minlength=nw * NQ))))[cellid]
        s1[ew2, eq2, pos] = (es[ord2] - eq2 * blkn).astype(np.int16)
        s2[ew2, eq2, pos] = (et2[ord2] - eq2 * cfg.t2blk).astype(np.int16)
        dm[ew2, eq2, pos] = (dl[ord2] % 128).astype(np.float32)

        pkt = np.zeros((nw, 128, cfg.pkt_w), np.int16)
        for w in range(nw):
            for q in range(NQ):
                pkt[w, :, q * ic:(q + 1) * ic] = _wrap_idx(s1[w, q])
                pkt[w, :, cfg.off_idx2 + q * ic:
                    cfg.off_idx2 + (q + 1) * ic] = _wrap_idx(s2[w, q])
            dmw = dm[w].reshape(NQ * C, 128)  # [t, s]
            pkt[w, :, cfg.off_dmod:cfg.off_dmod + NQ * C] = \
                dmw.T.astype(BF16).view(np.int16)
            xrow = dmw.reshape(-1).astype(BF16).view(np.int16)
            pkt[w, :, cfg.off_x:cfg.off_x + NQ * C * 128] = xrow[None, :]

        gid = p * ln + np.arange(cfg.t2ln, dtype=np.int32)
        gid[ln:] = 0
        er_gidx = gid.reshape(nw, 128).T.astype(np.int32)  # [128, nw]
        in_maps.append(dict(
            featT=featT, w1f=w1f_b, w2f=w2f_b, iota=iota_rep, iotap=iota_p,
            ident=ident, b1=b1_rep, b2=b2_rep, pw=pW_rep, pb=pb_t,
            pkt=pkt, er_gidx=er_gidx,
        ))
    return cfg, in_maps


def build(cfg: Cfg, stop_after: int = 99, parts: int = 99, repeat: int = 1):
    dt = mybir.dt
    nc = bacc.Bacc("TRN2", target_bir_lowering=False, debug=False,
                   num_devices=NCORES, dynamic_dma_scratch_size=SCRATCH,
                   num_swdge_queues=NQUEUES)
    ap = {}
    def inp(name, shape, dtype):
        ap[name] = nc.dram_tensor(name, shape, dtype, kind="ExternalInput").ap()
    inp("featT", [128, cfg.npad], dt.bfloat16)
    inp("w1f", [128, 272], dt.bfloat16)
    inp("w2f", [128, 2, 66], dt.bfloat16)
    inp("iota", [128, 128], dt.bfloat16)
    inp("iotap", [128, 1], dt.bfloat16)
    inp("ident", [128, 128], dt.bfloat16)
    inp("b1", [128, F1], dt.float32)
    inp("b2", [128, OUT_F], dt.float32)
    inp("pw", [128, OUT_F], dt.float32)
    inp("pb", [128, 1], dt.float32)
    inp("pkt", [cfg.nw, 128, cfg.pkt_w], dt.int16)
    inp("er_gidx", [128, cfg.nw], dt.int32)
    out_core = nc.dram_tensor("out_core", [128, cfg.nw], dt.float32,
                              kind="ExternalOutput").ap()

    t1 = nc.dram_tensor("t1", [cfg.npad, T1W], dt.bfloat16, kind="Internal").ap()
    er1f = nc.dram_tensor("er1f", [cfg.npad, 8], dt.float32, kind="Internal").ap()
    t2_loc = nc.dram_tensor("t2l", [cfg.t2ln, T2W], dt.bfloat16, kind="Internal").ap()
    t2_full = nc.dram_tensor("t2f", [NCORES * cfg.t2ln, T2W], dt.bfloat16,
                             kind="Internal", addr_space="Shared").ap()

    C = cfg.c
    ic = cfg.ic
    NCH = NQ * C  # chunks per window
    nblk_rows = [cfg.blkn] * (NQ - 1) + [cfg.npad - (NQ - 1) * cfg.blkn]

    with tile.TileContext(nc) as tc:
        with tc.tile_pool(name="persist", bufs=1) as pp:
            out_acc = pp.tile([128, cfg.nw], dt.float32, tag="out_acc")
            nc.gpsimd.memset(out_acc[:], 0.0)
            w1f_t = pp.tile([128, 272], dt.bfloat16, tag="w1f")
            nc.sync.dma_start(w1f_t[:], ap["w1f"][:])
            w2f_t = pp.tile([128, 2, 66], dt.bfloat16, tag="w2f")
            nc.sync.dma_start(w2f_t[:], ap["w2f"][:])
            iota_t = pp.tile([128, 128], dt.bfloat16, tag="iota")
            nc.sync.dma_start(iota_t[:], ap["iota"][:])
            iotap_t = pp.tile([128, 1], dt.bfloat16, tag="iotap")
            nc.sync.dma_start(iotap_t[:], ap["iotap"][:])
            ident_t = pp.tile([128, 128], dt.bfloat16, tag="ident")
            nc.sync.dma_start(ident_t[:], ap["ident"][:])
            b1_t = pp.tile([128, F1], dt.float32, tag="b1")
            nc.sync.dma_start(b1_t[:], ap["b1"][:])
            b2_t = pp.tile([128, OUT_F], dt.float32, tag="b2")
            nc.sync.dma_start(b2_t[:], ap["b2"][:])
            pw_t = pp.tile([128, OUT_F], dt.float32, tag="pw")
            nc.sync.dma_start(pw_t[:], ap["pw"][:])
            pb_t = pp.tile([128, 1], dt.float32, tag="pb")
            nc.sync.dma_start(pb_t[:], ap["pb"][:])
            gidx_t = pp.tile([128, cfg.nw], dt.int32, tag="gidx")
            nc.sync.dma_start(gidx_t[:], ap["er_gidx"][:])
            er1b_t = pp.tile([128, cfg.nw, 8], dt.bfloat16, tag="er1b")
            er2b_t = pp.tile([128, cfg.nw, 1], dt.bfloat16, tag="er2b")

            for _rep in range(repeat):
              # ---------- phase 0: fc1 over all nodes -> t1, er1f ----------
              with (
                  tc.tile_pool(name="p0", bufs=2) as p0,
                  tc.tile_pool(name="p0ps", bufs=4, space="PSUM") as p0ps,
              ):
                  for b in range(cfg.nbatch):
                      fsl = p0.tile([128, 1024], dt.bfloat16, tag="fsl")
                      nc.sync.dma_start(fsl[:], ap["featT"][:, b * 1024:(b + 1) * 1024])
                      stg = p0.tile([128, 8, T1W], dt.bfloat16, tag="stg")
                      nc.gpsimd.memset(stg[:, :, F1 + 16:T1W], 0.0)
                      stg_er = p0.tile([128, 8, 8], dt.float32, tag="stg_er")
                      for c in range(8):
                          ps = p0ps.tile([128, 272], dt.float32, space="PSUM", tag="ps")
                          nc.tensor.matmul(out=ps[:], lhsT=fsl[:, c * 128:(c + 1) * 128],
                                           rhs=w1f_t[:], start=True, stop=True)
                          if c % 2 == 0:
                              nc.scalar.activation(stg[:, c, 0:F1], ps[:, 0:F1],
                                                   mybir.ActivationFunctionType.Copy)
                          else:
                              nc.vector.tensor_copy(stg[:, c, 0:F1], ps[:, 0:F1])
                          nc.vector.tensor_copy(
                              stg[:, c, :].bitcast(dt.float32)[:, F1 // 2:F1 // 2 + 8],
                              ps[:, F1:F1 + 8])
                          nc.vector.tensor_copy(stg_er[:, c, :], ps[:, F1 + 8:F1 + 16])
                      nc.sync.dma_start(
                          t1[b * 1024:(b + 1) * 1024, :].rearrange(
                              "(c p) e -> p c e", p=128), stg[:])
                      nc.sync.dma_start(
                          er1f[b * 1024:(b + 1) * 1024, :].rearrange(
                              "(c p) e -> p c e", p=128), stg_er[:])
              tc.strict_bb_all_engine_barrier()

              # ---------- phase 0b: local er1 -> SBUF (bf16) ----------
              # NOTE: indirect_dma_start ignores the out AP base offset on HW
              # ucode — gather into a fresh tile at offset 0, then copy.
              if stop_after >= 2:
                with tc.tile_pool(name="erb", bufs=4) as erb:
                  for k in range(cfg.nw):
                      ert = erb.tile([128, 8], dt.float32, tag="ert")
                      nc.gpsimd.indirect_dma_start(
                          out=ert[:], out_offset=None, in_=er1f[:],
                          in_offset=bass.IndirectOffsetOnAxis(
                              ap=gidx_t[:, k:k + 1], axis=0))
                      nc.vector.tensor_copy(er1b_t[:, k, :], ert[:])
                tc.strict_bb_all_engine_barrier()

              # ---------- shared conv machinery ----------
              def conv_window(w, t_src, erb_t, idx_off, fw, agg_cols, post,
                              cv, cvps, first):
                  """fw: feature cols (bf16) in gathered row; el f32 at
                  [fw/2 : fw/2+nh]; agg_cols = fw + nh."""
                  nh = agg_cols - fw
                  elem = T1W if fw == F1 else T2W
                  gt = cv.tile([128, NQ, C, elem], dt.bfloat16, tag=f"gt{elem}")
                  if first:
                      nc.gpsimd.memset(gt[:], 0.0)
                  pkt_t = cv.tile([128, cfg.pkt_w], dt.int16, tag="pkt")
                  nc.sync.dma_start(pkt_t[:], ap["pkt"][w, :, :])
                  for q in range(NQ):
                      nc.gpsimd.dma_gather(
                          gt[:, q, :, :], t_src[q],
                          pkt_t[:, idx_off + q * ic: idx_off + (q + 1) * ic],
                          C * 128, C * 128, elem, queue_num=q % NQUEUES)
                  if parts < 3:
                      return
                  dmod = pkt_t[:, cfg.off_dmod:cfg.off_dmod + NCH].bitcast(dt.bfloat16)
                  xblk = pkt_t[:, cfg.off_x:cfg.off_x + NCH * 128].bitcast(dt.bfloat16)
                  st = cv.tile([128, NCH, 128], dt.bfloat16, tag="st")
                  nc.vector.tensor_tensor(
                      out=st[:],
                      in0=dmod.unsqueeze(2).to_broadcast([128, NCH, 128]),
                      in1=iota_t[:].unsqueeze(1).to_broadcast([128, NCH, 128]),
                      op=mybir.AluOpType.is_equal)
                  stT = cv.tile([128, NCH, 128], dt.bfloat16, tag="stT")
                  nc.vector.tensor_tensor(
                      out=stT[:],
                      in0=xblk.rearrange("p (t s) -> p t s", s=128),
                      in1=iotap_t[:].unsqueeze(2).to_broadcast([128, NCH, 128]),
                      op=mybir.AluOpType.is_equal)
                  if parts < 4:
                      return
                  ere = cvps.tile([128, NCH, nh], dt.float32, space="PSUM", tag="ere")
                  for t in range(NCH):
                      nc.tensor.matmul(out=ere[:, t, :], lhsT=stT[:, t, :],
                                       rhs=erb_t[:, w, :], start=True, stop=True)
                  el = gt[:].rearrange("p q c e -> p (q c) e").bitcast(
                      dt.float32)[:, :, fw // 2:fw // 2 + nh]
                  v = cv.tile([128, NCH, nh], dt.float32, tag="v")
                  nc.vector.tensor_tensor(out=v[:], in0=el, in1=ere[:],
                                          op=mybir.AluOpType.add)
                  e1 = cv.tile([128, NCH, nh], dt.float32, tag="e1")
                  nc.scalar.activation(e1[:], v[:],
                                       mybir.ActivationFunctionType.Exp)
                  e2 = cv.tile([128, NCH, nh], dt.float32, tag="e2")
                  nc.scalar.activation(e2[:], v[:],
                                       mybir.ActivationFunctionType.Exp, scale=NEG)
                  ee = cv.tile([128, NCH, nh], dt.bfloat16, tag="ee")
                  nc.vector.tensor_tensor(out=ee[:], in0=e1[:], in1=e2[:],
                                          op=mybir.AluOpType.max)
                  if parts < 5:
                      return
                  msg = cv.tile([128, NCH, agg_cols], dt.bfloat16, tag="msg")
                  gtf = gt[:].rearrange("p q c e -> p (q c) e")
                  nc.vector.tensor_tensor(
                      out=msg[:, :, 0:fw].rearrange("p c (h f) -> p c h f", h=nh),
                      in0=gtf[:, :, 0:fw].rearrange("p c (h f) -> p c h f", h=nh),
                      in1=ee[:].unsqueeze(3).to_broadcast(
                          [128, NCH, nh, fw // nh]),
                      op=mybir.AluOpType.mult)
                  nc.vector.tensor_copy(msg[:, :, fw:agg_cols], ee[:])
                  if parts < 6:
                      return
                  agg = cvps.tile([128, agg_cols], dt.float32, space="PSUM", tag="agg")
                  for t in range(NCH):
                      nc.tensor.matmul(out=agg[:], lhsT=st[:, t, :],
                                       rhs=msg[:, t, :],
                                       start=(t == 0), stop=(t == NCH - 1))
                  if parts < 7:
                      return
                  post(w, agg, cv, cvps)

              def post1(w, agg, cv, cvps):
                  dmx = cv.tile([128, HEADS], dt.float32, tag="dmx")
                  nc.vector.tensor_scalar(dmx[:], agg[:, F1:F1 + HEADS], 1e-20, None,
                                          mybir.AluOpType.max)
                  rec = cv.tile([128, HEADS], dt.float32, tag="rec")
                  nc.vector.reciprocal(rec[:], dmx[:])
                  o1 = cv.tile([128, F1], dt.float32, tag="o1")
                  nc.vector.tensor_tensor(
                      out=o1[:].rearrange("p (h f) -> p h f", h=HEADS),
                      in0=agg[:, 0:F1].rearrange("p (h f) -> p h f", h=HEADS),
                      in1=rec[:].unsqueeze(2).to_broadcast([128, HEADS, HID]),
                      op=mybir.AluOpType.mult)
                  nc.vector.tensor_tensor(out=o1[:], in0=o1[:], in1=b1_t[:],
                                          op=mybir.AluOpType.add)
                  xw = cv.tile([128, F1], dt.bfloat16, tag="xw")
                  nc.vector.tensor_scalar(xw[:], o1[:], 0.0, None,
                                          mybir.AluOpType.max)
                  if parts < 8:
                      return
                  xt = cv.tile([128, 2, 128], dt.bfloat16, tag="xt")
                  for h in range(2):
                      tp = cvps.tile([128, 128], dt.bfloat16, space="PSUM", tag="tp")
                      nc.tensor.transpose(
                          out=tp[:], in_=xw[:, h * 128:(h + 1) * 128],
                          identity=ident_t[:])
                      nc.vector.tensor_copy(xt[:, h, :], tp[:])
                  f2 = cvps.tile([128, 66], dt.float32, space="PSUM", tag="f2")
                  for h in range(2):
                      nc.tensor.matmul(out=f2[:], lhsT=xt[:, h, :], rhs=w2f_t[:, h, :],
                                       start=(h == 0), stop=(h == 1))
                  stg2 = cv.tile([128, T2W], dt.bfloat16, tag="stg2")
                  nc.gpsimd.memset(stg2[:, OUT_F + 2:T2W], 0.0)
                  nc.vector.tensor_copy(stg2[:, 0:OUT_F], f2[:, 0:OUT_F])
                  nc.vector.tensor_copy(
                      stg2[:].bitcast(dt.float32)[:, OUT_F // 2:OUT_F // 2 + 1],
                      f2[:, OUT_F:OUT_F + 1])
                  nc.vector.tensor_copy(er2b_t[:, w, :], f2[:, OUT_F + 1:OUT_F + 2])
                  nc.sync.dma_start(t2_loc[w * 128:(w + 1) * 128, :], stg2[:])

              def post2(w, agg, cv, cvps):
                  dmx = cv.tile([128, 1], dt.float32, tag="dmx2")
                  nc.vector.tensor_scalar(dmx[:], agg[:, OUT_F:OUT_F + 1], 1e-20, None,
                                          mybir.AluOpType.max)
                  rec = cv.tile([128, 1], dt.float32, tag="rec2")
                  nc.vector.reciprocal(rec[:], dmx[:])
                  o2 = cv.tile([128, OUT_F], dt.float32, tag="o2")
                  nc.vector.tensor_tensor(
                      out=o2[:], in0=agg[:, 0:OUT_F],
                      in1=rec[:].to_broadcast([128, OUT_F]),
                      op=mybir.AluOpType.mult)
                  nc.vector.tensor_tensor(out=o2[:], in0=o2[:], in1=b2_t[:],
                                          op=mybir.AluOpType.add)
                  nc.vector.tensor_scalar(o2[:], o2[:], 0.0, None,
                                          mybir.AluOpType.max)
                  nc.vector.tensor_tensor(out=o2[:], in0=o2[:], in1=pw_t[:],
                                          op=mybir.AluOpType.mult)
                  nc.vector.tensor_reduce(out=out_acc[:, w:w + 1], in_=o2[:],
                                          axis=mybir.AxisListType.X,
                                          op=mybir.AluOpType.add)

              # ---------- conv1 + fc2 per window ----------
              t1_blocks = [t1[q * cfg.blkn:q * cfg.blkn + nblk_rows[q], :]
                           for q in range(NQ)]
              if stop_after >= 3:
                  with (
                      tc.tile_pool(name="cv1", bufs=2) as cv,
                      tc.tile_pool(name="cv1ps", bufs=2, space="PSUM") as cvps,
                  ):
                      for w in range(cfg.nw):
                          conv_window(w, t1_blocks, er1b_t, 0, F1, F1 + HEADS,
                                      post1, cv, cvps, first=(w < 2))
                  tc.strict_bb_all_engine_barrier()

              if stop_after >= 4:
                  nc.gpsimd.collective_compute(
                      "AllGather", mybir.AluOpType.bypass,
                      replica_groups=[list(range(NCORES))],
                      ins=[t2_loc[:]], outs=[t2_full[:]])
                  tc.strict_bb_all_engine_barrier()

              if stop_after >= 5:
                  t2_blocks = [t2_full[q * cfg.t2blk:(q + 1) * cfg.t2blk, :]
                               for q in range(NQ)]
                  with (
                      tc.tile_pool(name="cv2", bufs=2) as cv,
                      tc.tile_pool(name="cv2ps", bufs=2, space="PSUM") as cvps,
                  ):
                      for w in range(cfg.nw):
                          conv_window(w, t2_blocks, er2b_t, cfg.off_idx2, OUT_F,
                                      OUT_F + 1, post2, cv, cvps, first=(w < 2))
                      sg = cv.tile([128, cfg.nw], dt.float32, tag="sg")
                      nc.scalar.activation(sg[:], out_acc[:],
                                           mybir.ActivationFunctionType.Sigmoid,
                                           bias=pb_t[:])
                      nc.sync.dma_start(out_core[:], sg[:])
              elif stop_after >= 3:
                  nc.sync.dma_start(out_core[:], out_acc[:])
    nc.compile()
    return nc


_CACHE = {}


def _get_nc(cfg):
    if cfg not in _CACHE:
        _CACHE[cfg] = build(cfg)
    return _CACHE[cfg]


def kernel(**inputs) -> np.ndarray:
    cfg, in_maps = prep(inputs)
    nc = _get_nc(cfg)
    res = run_bass_kernel_spmd(nc, in_maps, core_ids=list(range(NCORES)))
    n = cfg.n
    ln = cfg.ln
    out = np.empty(n, np.float32)
    for p in range(NCORES):
        oc = np.asarray(res.results[p]["out_core"])  # [128, nw]
        flat = oc.T.reshape(-1)[:ln]  # node = 128*w + part
        out[p * ln:(p + 1) * ln] = flat
    return out



# revision 3
# speedup vs baseline: 160.0353x; 160.0353x over previous
"""GAT 2-layer model on 8 Trainium2 NeuronCores (Bass/Tile) — v2.

Strategy (vs v1): nodes dst-sharded across 8 cores; edges of core p grouped
by (dst-window of 128, src-block of 25k), cell-packed into C chunks of 128
slots with padding at each cell's tail. fc1 replicated: each core computes
the full [N,272] fc1+att table in its HBM (t1, 768B bf16 rows: 256 feat |
8 f32 el | pad) plus a compact er table (er1f). Per window: one dma_gather
per src-block with NEGATIVE trailing pad indices (descriptors skipped);
el comes with the gathered rows; er[dst] delivered on-chip: a transposed
one-hot stT (built on DVE from a host-packed broadcast dmod table) feeds
tiny PE matmuls that scatter the window's 128 er values to edge slots —
no per-edge er gather, no er_loc tables. leaky-relu+exp done as
max(exp(v), exp(0.2 v)) so ACT only ever runs Exp (no act-table reloads);
relu on DVE; sigmoid once at the end. Segment-sum via one-hot matmul into
PSUM. conv2 repeats the machinery on the AllGathered t2 table.
"""
import sys

for _p in ("/opt/trn_rl_repo",):
    if _p not in sys.path:
        sys.path.insert(0, _p)

import math
from dataclasses import dataclass

import numpy as np
import ml_dtypes

import concourse.bass as bass
import concourse.bacc as bacc
import concourse.mybir as mybir
import concourse.tile as tile
from concourse.bass_utils import run_bass_kernel_spmd

BF16 = ml_dtypes.bfloat16
NCORES = 8
NQ = 4  # src blocks
NEG = 0.2

IN_F = 128
HID = 32
HEADS = 8
OUT_F = 64
F1 = HEADS * HID          # 256
T1W = 384                 # bf16 cols per T1 row: 256 feat | 16 (el f32) | pad
T2W = 128                 # bf16 cols per T2 row: 64 feat | 2 (el2 f32) | pad
SCRATCH = 16384           # dynamic DMA scratch (default)
NQUEUES = 4


@dataclass(frozen=True)
class Cfg:
    n: int
    e: int
    c: int  # slot chunks (x128) per (window, src-block) cell

    @property
    def ln(self):
        return self.n // NCORES

    @property
    def nw(self):
        return math.ceil(self.ln / 128)

    @property
    def t2ln(self):
        return self.nw * 128

    @property
    def blkn(self):
        return self.n // NQ

    @property
    def t2blk(self):
        return 2 * self.t2ln

    @property
    def npad(self):
        return 1024 * math.ceil(self.n / 1024)

    @property
    def nbatch(self):
        return self.npad // 1024

    @property
    def ic(self):  # int16 idx cols per (window, q) cell
        return self.c * 128 // 16

    @property
    def pkt_w(self):  # int16 cols of the per-window packet
        # idx1[NQ][ic] | idx2[NQ][ic] | dmod[NQ*C] | X[NQ*C*128]
        w = 2 * NQ * self.ic + NQ * self.c + NQ * self.c * 128
        return 16 * math.ceil(w / 16)

    @property
    def off_idx2(self):
        return NQ * self.ic

    @property
    def off_dmod(self):
        return 2 * NQ * self.ic

    @property
    def off_x(self):
        return 2 * NQ * self.ic + NQ * self.c


def _fold(al, heads, hid):
    a = np.zeros((heads * hid, heads), np.float32)
    for h in range(heads):
        a[h * hid:(h + 1) * hid, h] = al[h]
    return a


def _wrap_idx(flat):
    """[n] int -> [128, n//16] int16: wrapped in 16 partitions, replicated 8x."""
    n = len(flat)
    w = np.asarray(flat, np.int16).reshape(n // 16, 16).T
    return np.tile(w, (8, 1))


def prep(inputs, cfg: Cfg | None = None):
    """Host-side: fold weights, pack per-core edge schedules."""
    feats = np.asarray(inputs["features"], np.float32)
    src = np.asarray(inputs["src"], np.int64)
    dst = np.asarray(inputs["dst"], np.int64)
    n, e = feats.shape[0], src.shape[0]

    W1f = np.concatenate(
        [inputs["W1"],
         inputs["W1"] @ _fold(np.asarray(inputs["al1"]), HEADS, HID),
         inputs["W1"] @ _fold(np.asarray(inputs["ar1"]), HEADS, HID)], axis=1
    ).astype(np.float32)  # [128, 272]
    W2f = np.concatenate(
        [inputs["W2"],
         inputs["W2"] @ _fold(np.asarray(inputs["al2"]), 1, OUT_F),
         inputs["W2"] @ _fold(np.asarray(inputs["ar2"]), 1, OUT_F)], axis=1
    ).astype(np.float32)  # [256, 66]

    ln = n // NCORES
    core = dst // ln
    nw = math.ceil(ln / 128)
    blkn = n // NQ
    w_of = (dst - core * ln) // 128
    q_of = src // blkn
    cell = ((core * nw + w_of) * NQ + q_of).astype(np.int64)
    counts = np.bincount(cell, minlength=NCORES * nw * NQ)
    c_need = math.ceil(counts.max() / 128)
    if cfg is None:
        cfg = Cfg(n=n, e=e, c=int(c_need))
    assert counts.max() <= cfg.c * 128, (counts.max(), cfg.c)
    C = cfg.c

    t2row_src = (src // ln) * cfg.t2ln + (src % ln)

    order = np.lexsort((q_of, w_of, core))
    src_s, dst_s = src[order], dst[order]
    core_s, w_s, q_s = core[order], w_of[order], q_of[order]
    t2src_s = t2row_src[order]

    in_maps = []
    featT = np.zeros((128, cfg.npad), BF16)
    featT[:, :n] = feats.T.astype(BF16)
    iota_rep = np.tile(np.arange(128, dtype=np.float32), (128, 1)).astype(BF16)
    iota_p = np.arange(128, dtype=np.float32)[:, None].astype(BF16)  # [128,1]
    ident = np.eye(128, dtype=np.float32).astype(BF16)
    w1f_b = W1f.astype(BF16)
    w2f_b = np.zeros((128, 2, 66), BF16)
    w2f_b[:, 0, :] = W2f[:128].astype(BF16)
    w2f_b[:, 1, :] = W2f[128:].astype(BF16)
    b1_rep = np.tile(np.asarray(inputs["b1"], np.float32)[None, :], (128, 1))
    b2_rep = np.tile(np.asarray(inputs["b2"], np.float32)[None, :], (128, 1))
    pW_rep = np.tile(np.asarray(inputs["pW"], np.float32)[:, 0][None, :], (128, 1))
    pb_t = np.full((128, 1), float(np.asarray(inputs["pb"])[0]), np.float32)

    ic = cfg.ic
    for p in range(NCORES):
        sel = core_s == p
        es, ed, ew, eq, et2 = src_s[sel], dst_s[sel], w_s[sel], q_s[sel], t2src_s[sel]
        dl = ed - p * ln
        s1 = np.zeros((nw, NQ, C * 128), np.int16)
        s2 = np.zeros((nw, NQ, C * 128), np.int16)
        dm = np.full((nw, NQ, C * 128), 200.0, np.float32)
        ord2 = np.lexsort((eq, ew))
        ew2, eq2 = ew[ord2], eq[ord2]
        cellid = ew2 * NQ + eq2
        pos = np.arange(len(cellid)) - np.concatenate(
            ([0], np.cumsum(np.bincount(cellid, minlength=nw * NQ))))[cellid]
        s1[ew2, eq2, pos] = (es[ord2] - eq2 * blkn).astype(np.int16)
        s2[ew2, eq2, pos] = (et2[ord2] - eq2 * cfg.t2blk).astype(np.int16)
        dm[ew2, eq2, pos] = (dl[ord2] % 128).astype(np.float32)

        pkt = np.zeros((nw, 128, cfg.pkt_w), np.int16)
        for w in range(nw):
            for q in range(NQ):
                pkt[w, :, q * ic:(q + 1) * ic] = _wrap_idx(s1[w, q])
                pkt[w, :, cfg.off_idx2 + q * ic:
                    cfg.off_idx2 + (q + 1) * ic] = _wrap_idx(s2[w, q])
            dmw = dm[w].reshape(NQ * C, 128)  # [t, s]
            pkt[w, :, cfg.off_dmod:cfg.off_dmod + NQ * C] = \
                dmw.T.astype(BF16).view(np.int16)
            xrow = dmw.reshape(-1).astype(BF16).view(np.int16)
            pkt[w, :, cfg.off_x:cfg.off_x + NQ * C * 128] = xrow[None, :]

        gid = p * ln + np.arange(cfg.t2ln, dtype=np.int32)
        gid[ln:] = 0
        er_gidx = gid.reshape(nw, 128).T.astype(np.int32)  # [128, nw]
        in_maps.append(dict(
            featT=featT, w1f=w1f_b, w2f=w2f_b, iota=iota_rep, iotap=iota_p,
            ident=ident, b1=b1_rep, b2=b2_rep, pw=pW_rep, pb=pb_t,
            pkt=pkt, er_gidx=er_gidx,
        ))
    return cfg, in_maps


def build(cfg: Cfg, stop_after: int = 99, parts: int = 99, repeat: int = 1,
          sim: bool = False):
    dt = mybir.dt
    nc = bacc.Bacc("TRN2", target_bir_lowering=False, debug=False,
                   num_devices=NCORES, dynamic_dma_scratch_size=SCRATCH,
                   num_swdge_queues=NQUEUES)
    ap = {}
    def inp(name, shape, dtype):
        ap[name] = nc.dram_tensor(name, shape, dtype, kind="ExternalInput").ap()
    inp("featT", [128, cfg.npad], dt.bfloat16)
    inp("w1f", [128, 272], dt.bfloat16)
    inp("w2f", [128, 2, 66], dt.bfloat16)
    inp("iota", [128, 128], dt.bfloat16)
    inp("iotap", [128, 1], dt.bfloat16)
    inp("ident", [128, 128], dt.bfloat16)
    inp("b1", [128, F1], dt.float32)
    inp("b2", [128, OUT_F], dt.float32)
    inp("pw", [128, OUT_F], dt.float32)
    inp("pb", [128, 1], dt.float32)
    inp("pkt", [cfg.nw, 128, cfg.pkt_w], dt.int16)
    inp("er_gidx", [128, cfg.nw], dt.int32)
    out_core = nc.dram_tensor("out_core", [128, cfg.nw], dt.float32,
                              kind="ExternalOutput").ap()

    t1 = nc.dram_tensor("t1", [cfg.npad, T1W], dt.bfloat16, kind="Internal").ap()
    er1f = nc.dram_tensor("er1f", [cfg.npad, 8], dt.float32, kind="Internal").ap()
    t2_loc = nc.dram_tensor("t2l", [cfg.t2ln, T2W], dt.bfloat16, kind="Internal").ap()
    t2_full = nc.dram_tensor("t2f", [NCORES * cfg.t2ln, T2W], dt.bfloat16,
                             kind="Internal", addr_space="Shared").ap()

    C = cfg.c
    ic = cfg.ic
    NCH = NQ * C  # chunks per window
    nblk_rows = [cfg.blkn] * (NQ - 1) + [cfg.npad - (NQ - 1) * cfg.blkn]

    with tile.TileContext(nc) as tc:
        with tc.tile_pool(name="persist", bufs=1) as pp:
            out_acc = pp.tile([128, cfg.nw], dt.float32, tag="out_acc")
            nc.gpsimd.memset(out_acc[:], 0.0)
            w1f_t = pp.tile([128, 272], dt.bfloat16, tag="w1f")
            nc.sync.dma_start(w1f_t[:], ap["w1f"][:])
            w2f_t = pp.tile([128, 2, 66], dt.bfloat16, tag="w2f")
            nc.sync.dma_start(w2f_t[:], ap["w2f"][:])
            iota_t = pp.tile([128, 128], dt.bfloat16, tag="iota")
            nc.sync.dma_start(iota_t[:], ap["iota"][:])
            iotap_t = pp.tile([128, 1], dt.bfloat16, tag="iotap")
            nc.sync.dma_start(iotap_t[:], ap["iotap"][:])
            ident_t = pp.tile([128, 128], dt.bfloat16, tag="ident")
            nc.sync.dma_start(ident_t[:], ap["ident"][:])
            b1_t = pp.tile([128, F1], dt.float32, tag="b1")
            nc.sync.dma_start(b1_t[:], ap["b1"][:])
            b2_t = pp.tile([128, OUT_F], dt.float32, tag="b2")
            nc.sync.dma_start(b2_t[:], ap["b2"][:])
            pw_t = pp.tile([128, OUT_F], dt.float32, tag="pw")
            nc.sync.dma_start(pw_t[:], ap["pw"][:])
            pb_t = pp.tile([128, 1], dt.float32, tag="pb")
            nc.sync.dma_start(pb_t[:], ap["pb"][:])
            gidx_t = pp.tile([128, cfg.nw], dt.int32, tag="gidx")
            nc.sync.dma_start(gidx_t[:], ap["er_gidx"][:])
            er1b_t = pp.tile([128, cfg.nw, 8], dt.bfloat16, tag="er1b")
            er2b_t = pp.tile([128, cfg.nw, 1], dt.bfloat16, tag="er2b")

            for _rep in range(repeat):
              # ---------- phase 0: fc1 over all nodes -> t1, er1f ----------
              with (
                  tc.tile_pool(name="p0", bufs=2) as p0,
                  tc.tile_pool(name="p0ps", bufs=4, space="PSUM") as p0ps,
              ):
                  for b in range(cfg.nbatch):
                      fsl = p0.tile([128, 1024], dt.bfloat16, tag="fsl")
                      nc.sync.dma_start(fsl[:], ap["featT"][:, b * 1024:(b + 1) * 1024])
                      stg = p0.tile([128, 8, T1W], dt.bfloat16, tag="stg")
                      nc.gpsimd.memset(stg[:, :, F1 + 16:T1W], 0.0)
                      stg_er = p0.tile([128, 8, 8], dt.float32, tag="stg_er")
                      for c in range(8):
                          ps = p0ps.tile([128, 272], dt.float32, space="PSUM", tag="ps")
                          nc.tensor.matmul(out=ps[:], lhsT=fsl[:, c * 128:(c + 1) * 128],
                                           rhs=w1f_t[:], start=True, stop=True)
                          if c % 2 == 0:
                              nc.scalar.activation(stg[:, c, 0:F1], ps[:, 0:F1],
                                                   mybir.ActivationFunctionType.Copy)
                          else:
                              nc.vector.tensor_copy(stg[:, c, 0:F1], ps[:, 0:F1])
                          nc.vector.tensor_copy(
                              stg[:, c, :].bitcast(dt.float32)[:, F1 // 2:F1 // 2 + 8],
                              ps[:, F1:F1 + 8])
                          nc.vector.tensor_copy(stg_er[:, c, :], ps[:, F1 + 8:F1 + 16])
                      nc.sync.dma_start(
                          t1[b * 1024:(b + 1) * 1024, :].rearrange(
                              "(c p) e -> p c e", p=128), stg[:])
                      nc.sync.dma_start(
                          er1f[b * 1024:(b + 1) * 1024, :].rearrange(
                              "(c p) e -> p c e", p=128), stg_er[:])
              tc.strict_bb_all_engine_barrier()

              # ---------- phase 0b: local er1 -> SBUF (bf16) ----------
              # NOTE: indirect_dma_start ignores the out AP base offset on HW
              # ucode — gather into a fresh tile at offset 0, then copy.
              if stop_after >= 2:
                with tc.tile_pool(name="erb", bufs=4) as erb:
                  for k in range(cfg.nw):
                      ert = erb.tile([128, 8], dt.float32, tag="ert")
                      nc.gpsimd.indirect_dma_start(
                          out=ert[:], out_offset=None, in_=er1f[:],
                          in_offset=bass.IndirectOffsetOnAxis(
                              ap=gidx_t[:, k:k + 1], axis=0))
                      nc.vector.tensor_copy(er1b_t[:, k, :], ert[:])
                tc.strict_bb_all_engine_barrier()

              # ---------- shared conv machinery ----------
              def conv_window(w, t_src, erb_t, idx_off, fw, agg_cols, post,
                              cv, cvps, first):
                  """fw: feature cols (bf16) in gathered row; el f32 at
                  [fw/2 : fw/2+nh]; agg_cols = fw + nh."""
                  nh = agg_cols - fw
                  elem = T1W if fw == F1 else T2W
                  gt = cv.tile([128, NQ, C, elem], dt.bfloat16, tag=f"gt{elem}")
                  if first:
                      nc.gpsimd.memset(gt[:], 0.0)
                  pkt_t = cv.tile([128, cfg.pkt_w], dt.int16, tag="pkt")
                  nc.sync.dma_start(pkt_t[:], ap["pkt"][w, :, :])
                  for q in range(NQ):
                      nc.gpsimd.dma_gather(
                          gt[:, q, :, :], t_src[q],
                          pkt_t[:, idx_off + q * ic: idx_off + (q + 1) * ic],
                          C * 128, C * 128, elem, queue_num=q % NQUEUES)
                  if parts < 3:
                      return
                  dmod = pkt_t[:, cfg.off_dmod:cfg.off_dmod + NCH].bitcast(dt.bfloat16)
                  xblk = pkt_t[:, cfg.off_x:cfg.off_x + NCH * 128].bitcast(dt.bfloat16)
                  st = cv.tile([128, NCH, 128], dt.bfloat16, tag="st")
                  nc.vector.tensor_tensor(
                      out=st[:],
                      in0=dmod.unsqueeze(2).to_broadcast([128, NCH, 128]),
                      in1=iota_t[:].unsqueeze(1).to_broadcast([128, NCH, 128]),
                      op=mybir.AluOpType.is_equal)
                  stT = cv.tile([128, NCH, 128], dt.bfloat16, tag="stT")
                  nc.vector.tensor_tensor(
                      out=stT[:],
                      in0=xblk.rearrange("p (t s) -> p t s", s=128),
                      in1=iotap_t[:].unsqueeze(2).to_broadcast([128, NCH, 128]),
                      op=mybir.AluOpType.is_equal)
                  if parts < 4:
                      return
                  ere = cvps.tile([128, NCH, nh], dt.float32, space="PSUM", tag="ere")
                  for t in range(NCH):
                      nc.tensor.matmul(out=ere[:, t, :], lhsT=stT[:, t, :],
                                       rhs=erb_t[:, w, :], start=True, stop=True)
                  el = gt[:].rearrange("p q c e -> p (q c) e").bitcast(
                      dt.float32)[:, :, fw // 2:fw // 2 + nh]
                  v = cv.tile([128, NCH, nh], dt.float32, tag="v")
                  nc.vector.tensor_tensor(out=v[:], in0=el, in1=ere[:],
                                          op=mybir.AluOpType.add)
                  e1 = cv.tile([128, NCH, nh], dt.float32, tag="e1")
                  nc.scalar.activation(e1[:], v[:],
                                       mybir.ActivationFunctionType.Exp)
                  e2 = cv.tile([128, NCH, nh], dt.float32, tag="e2")
                  nc.scalar.activation(e2[:], v[:],
                                       mybir.ActivationFunctionType.Exp, scale=NEG)
                  ee = cv.tile([128, NCH, nh], dt.bfloat16, tag="ee")
                  nc.vector.tensor_tensor(out=ee[:], in0=e1[:], in1=e2[:],
                                          op=mybir.AluOpType.max)
                  if parts < 5:
                      return
                  msg = cv.tile([128, NCH, agg_cols], dt.bfloat16, tag="msg")
                  gtf = gt[:].rearrange("p q c e -> p (q c) e")
                  nc.vector.tensor_tensor(
                      out=msg[:, :, 0:fw].rearrange("p c (h f) -> p c h f", h=nh),
                      in0=gtf[:, :, 0:fw].rearrange("p c (h f) -> p c h f", h=nh),
                      in1=ee[:].unsqueeze(3).to_broadcast(
                          [128, NCH, nh, fw // nh]),
                      op=mybir.AluOpType.mult)
                  nc.vector.tensor_copy(msg[:, :, fw:agg_cols], ee[:])
                  if parts < 6:
                      return
                  agg = cvps.tile([128, agg_cols], dt.float32, space="PSUM", tag="agg")
                  for t in range(NCH):
                      nc.tensor.matmul(out=agg[:], lhsT=st[:, t, :],
                                       rhs=msg[:, t, :],
                                       start=(t == 0), stop=(t == NCH - 1))
                  if parts < 7:
                      return
                  post(w, agg, cv, cvps)

              def post1(w, agg, cv, cvps):
                  dmx = cv.tile([128, HEADS], dt.float32, tag="dmx")
                  nc.vector.tensor_scalar(dmx[:], agg[:, F1:F1 + HEADS], 1e-20, None,
                                          mybir.AluOpType.max)
                  rec = cv.tile([128, HEADS], dt.float32, tag="rec")
                  nc.vector.reciprocal(rec[:], dmx[:])
                  o1 = cv.tile([128, F1], dt.float32, tag="o1")
                  nc.vector.tensor_tensor(
                      out=o1[:].rearrange("p (h f) -> p h f", h=HEADS),
                      in0=agg[:, 0:F1].rearrange("p (h f) -> p h f", h=HEADS),
                      in1=rec[:].unsqueeze(2).to_broadcast([128, HEADS, HID]),
                      op=mybir.AluOpType.mult)
                  nc.vector.tensor_tensor(out=o1[:], in0=o1[:], in1=b1_t[:],
                                          op=mybir.AluOpType.add)
                  xw = cv.tile([128, F1], dt.bfloat16, tag="xw")
                  nc.vector.tensor_scalar(xw[:], o1[:], 0.0, None,
                                          mybir.AluOpType.max)
                  if parts < 8:
                      return
                  xt = cv.tile([128, 2, 128], dt.bfloat16, tag="xt")
                  for h in range(2):
                      tp = cvps.tile([128, 128], dt.bfloat16, space="PSUM", tag="tp")
                      nc.tensor.transpose(
                          out=tp[:], in_=xw[:, h * 128:(h + 1) * 128],
                          identity=ident_t[:])
                      nc.vector.tensor_copy(xt[:, h, :], tp[:])
                  f2 = cvps.tile([128, 66], dt.float32, space="PSUM", tag="f2")
                  for h in range(2):
                      nc.tensor.matmul(out=f2[:], lhsT=xt[:, h, :], rhs=w2f_t[:, h, :],
                                       start=(h == 0), stop=(h == 1))
                  stg2 = cv.tile([128, T2W], dt.bfloat16, tag="stg2")
                  nc.gpsimd.memset(stg2[:, OUT_F + 2:T2W], 0.0)
                  nc.vector.tensor_copy(stg2[:, 0:OUT_F], f2[:, 0:OUT_F])
                  nc.vector.tensor_copy(
                      stg2[:].bitcast(dt.float32)[:, OUT_F // 2:OUT_F // 2 + 1],
                      f2[:, OUT_F:OUT_F + 1])
                  nc.vector.tensor_copy(er2b_t[:, w, :], f2[:, OUT_F + 1:OUT_F + 2])
                  nc.sync.dma_start(t2_loc[w * 128:(w + 1) * 128, :], stg2[:])

              def post2(w, agg, cv, cvps):
                  dmx = cv.tile([128, 1], dt.float32, tag="dmx2")
                  nc.vector.tensor_scalar(dmx[:], agg[:, OUT_F:OUT_F + 1], 1e-20, None,
                                          mybir.AluOpType.max)
                  rec = cv.tile([128, 1], dt.float32, tag="rec2")
                  nc.vector.reciprocal(rec[:], dmx[:])
                  o2 = cv.tile([128, OUT_F], dt.float32, tag="o2")
                  nc.vector.tensor_tensor(
                      out=o2[:], in0=agg[:, 0:OUT_F],
                      in1=rec[:].to_broadcast([128, OUT_F]),
                      op=mybir.AluOpType.mult)
                  nc.vector.tensor_tensor(out=o2[:], in0=o2[:], in1=b2_t[:],
                                          op=mybir.AluOpType.add)
                  nc.vector.tensor_scalar(o2[:], o2[:], 0.0, None,
                                          mybir.AluOpType.max)
                  nc.vector.tensor_tensor(out=o2[:], in0=o2[:], in1=pw_t[:],
                                          op=mybir.AluOpType.mult)
                  nc.vector.tensor_reduce(out=out_acc[:, w:w + 1], in_=o2[:],
                                          axis=mybir.AxisListType.X,
                                          op=mybir.AluOpType.add)

              # ---------- conv1 + fc2 per window ----------
              t1_blocks = [t1[q * cfg.blkn:q * cfg.blkn + nblk_rows[q], :]
                           for q in range(NQ)]
              if stop_after >= 3:
                  with (
                      tc.tile_pool(name="cv1", bufs=2) as cv,
                      tc.tile_pool(name="cv1ps", bufs=2, space="PSUM") as cvps,
                  ):
                      for w in range(cfg.nw):
                          conv_window(w, t1_blocks, er1b_t, 0, F1, F1 + HEADS,
                                      post1, cv, cvps, first=(w < 2))
                  tc.strict_bb_all_engine_barrier()

              if stop_after >= 4:
                  if sim:
                      # TimelineSim can't model collectives: approximate the
                      # AllGather's local HBM write traffic with 8 DMA copies.
                      for pp_ in range(NCORES):
                          nc.sync.dma_start(
                              t2_full[pp_ * cfg.t2ln:(pp_ + 1) * cfg.t2ln, :],
                              t2_loc[:])
                  else:
                      nc.gpsimd.collective_compute(
                          "AllGather", mybir.AluOpType.bypass,
                          replica_groups=[list(range(NCORES))],
                          ins=[t2_loc[:]], outs=[t2_full[:]])
                  tc.strict_bb_all_engine_barrier()

              if stop_after >= 5:
                  t2_blocks = [t2_full[q * cfg.t2blk:(q + 1) * cfg.t2blk, :]
                               for q in range(NQ)]
                  with (
                      tc.tile_pool(name="cv2", bufs=2) as cv,
                      tc.tile_pool(name="cv2ps", bufs=2, space="PSUM") as cvps,
                  ):
                      for w in range(cfg.nw):
                          conv_window(w, t2_blocks, er2b_t, cfg.off_idx2, OUT_F,
                                      OUT_F + 1, post2, cv, cvps, first=(w < 2))
                      sg = cv.tile([128, cfg.nw], dt.float32, tag="sg")
                      nc.scalar.activation(sg[:], out_acc[:],
                                           mybir.ActivationFunctionType.Sigmoid,
                                           bias=pb_t[:])
                      nc.sync.dma_start(out_core[:], sg[:])
              elif stop_after >= 3:
                  nc.sync.dma_start(out_core[:], out_acc[:])
    nc.compile()
    return nc


_CACHE = {}


def _get_nc(cfg):
    if cfg not in _CACHE:
        _CACHE[cfg] = build(cfg)
    return _CACHE[cfg]


def kernel(**inputs) -> np.ndarray:
    cfg, in_maps = prep(inputs)
    nc = _get_nc(cfg)
    res = run_bass_kernel_spmd(nc, in_maps, core_ids=list(range(NCORES)))
    n = cfg.n
    ln = cfg.ln
    out = np.empty(n, np.float32)
    for p in range(NCORES):
        oc = np.asarray(res.results[p]["out_core"])  # [128, nw]
        flat = oc.T.reshape(-1)[:ln]  # node = 128*w + part
        out[p * ln:(p + 1) * ln] = flat
    return out

